# revision 34
# baseline (speedup 1.0000x reference)
"""Trainium2 Bass kernel for nn_EquivariantLayer (spectral equivariant layer).

Pipeline (per the reference):
  out1 = irfft2(downsample(rfft2(f)))                       [8,16,128,128]
  out2 = irfft2(einsum_c(downsample(rfft2(f)), K))          [8,32,128,128]
  K    = rfft2(D4_symmetrize(kernel))  -- purely real because the
         D4-symmetrized kernel is even in both axes.

All DFTs are expressed as matmuls against precomputed cos/sin matrices.
The Klein-subgroup part of the D4 symmetrization is absorbed by the cosine
transforms; the remaining transpose-average is done on the host (numpy).

Sharding (8 cores): core i owns batch b=i for the forward transform and
out1, and owns output channels d in [4i, 4i+4) for the kernel transform,
spectral einsum and out2.  The downsampled spectra F are exchanged with an
AllGather between the two phases.
"""
import os
import numpy as np

import concourse.bass as bass
import concourse.mybir as mybir
from concourse import bacc, tile
from concourse.bass_utils import run_bass_kernel_spmd

B, C1, C2, N1, N2, S = 8, 16, 32, 256, 128, 65
NCORES = 8
DL = C2 // NCORES          # 4 output channels per core
FW = 2 * S                 # 130: re|im packed along free dim
F32 = mybir.dt.float32
F32R = mybir.dt.float32r
BF16 = mybir.dt.bfloat16
USE_BF16 = os.environ.get("K_BF16", "0") == "1"

_PROGRAM = None            # (nc, const_arrays)
LAST_RESULT = None         # BassKernelResults of the most recent run


def _constants():
    """Host-side DFT matrices, laid out [partition, free] exactly as DMA'd."""
    r = np.arange(N2)
    k_r = np.where(r <= N2 // 2, r, r - N2)          # 0..64, -63..-1
    x = np.arange(N1)
    ang_x = 2 * np.pi * np.outer(k_r, x) / N1        # [128, 256]
    Xc = np.cos(ang_x) / 4.0
    Xs = np.sin(ang_x) / 4.0
    Xc[N2 // 2, :] = 0.0                             # zero x-nyquist row
    Xs[N2 // 2, :] = 0.0

    s = np.arange(S)
    y = np.arange(N1)
    ang_y = 2 * np.pi * np.outer(y, s) / N1          # [256, 65]
    Cy = np.cos(ang_y)
    Sy = np.sin(ang_y)
    Cy[:, S - 1] = 0.0                               # zero y-nyquist col
    Sy[:, S - 1] = 0.0

    n = np.arange(N2)
    E = np.cos(2 * np.pi * np.outer(n, n) / N2)      # [128,128] symmetric
    C65T = np.cos(2 * np.pi * np.outer(n, s) / N2)   # [128(v), 65(s)]

    Vc = np.cos(2 * np.pi * np.outer(n, n) / N2) / N2
    Vs = np.sin(2 * np.pi * np.outer(n, n) / N2) / N2

    w = np.full(S, 2.0)
    w[0] = 1.0
    w[S - 1] = 1.0
    ang_a = 2 * np.pi * np.outer(s, n) / N2
    A = (w[:, None] * np.cos(ang_a)) / N2            # [65,128]
    Bm = -(w[:, None] * np.sin(ang_a)) / N2

    XCS = np.concatenate([Xc.T, Xs.T], axis=1)         # [256, 256]
    R1 = np.concatenate([Cy, -Sy], axis=1)             # [256, 130]
    R2 = np.concatenate([-Sy, -Cy], axis=1)            # [256, 130]
    VCSA = np.concatenate([Vc, Vs], axis=1)            # [128, 256]
    VCSB = np.concatenate([-Vs, Vc], axis=1)           # [128, 256]
    # one [128, W] pack: all 128-row constants side by side
    segs128 = [("xcs0", XCS[0:128]), ("xcs1", XCS[128:256]),
               ("r10", R1[0:128]), ("r11", R1[128:256]),
               ("r20", R2[0:128]), ("r21", R2[128:256]),
               ("e", E), ("c65t", C65T), ("vcsa", VCSA), ("vcsb", VCSB)]
    off = {}
    cur = 0
    for nm, arr in segs128:
        off[nm] = cur
        cur += arr.shape[1]
    cpack = np.concatenate([a for _, a in segs128], axis=1)   # [128, cur]
    cpack65 = np.concatenate([A, Bm], axis=1)                 # [65, 256]
    cons = {"cpack": np.ascontiguousarray(cpack.astype(np.float32)),
            "cpack65": np.ascontiguousarray(cpack65.astype(np.float32))}
    return cons, off


def _build_program(comm=True, skip=()):
    """comm=False replaces the AllGather with equivalent-traffic local DMAs so
    the single-core TimelineSim cost model can run (used only for perf
    estimation, never for real results).  skip: subset of
    {"kpath","out1","einsum","inv2"} for differential cost analysis."""
    cons, coff = _constants()
    nc = bacc.Bacc("TRN2", num_devices=NCORES)

    f_in = nc.declare_dram_parameter("f_in", [C1, N1, N1], F32R, isOutput=False)
    krs_in = nc.declare_dram_parameter("krs_in", [C1, DL, N2, N2], F32, isOutput=False)
    cts = {
        k: nc.declare_dram_parameter("c_" + k, list(v.shape),
                                     F32R if k == "cpack" else F32,
                                     isOutput=False)
        for k, v in cons.items()
    }
    out1_t = nc.declare_dram_parameter("out1", [C1, N2, N2], F32, isOutput=True)
    out2_t = nc.declare_dram_parameter("out2", [B, DL, N2, N2], F32, isOutput=True)

    mm = None  # forward-declared for clarity inside helpers

    with tile.TileContext(nc) as tc:
        with (
            tc.tile_pool(name="const", bufs=1) as constp,
            tc.tile_pool(name="fbig", bufs=1) as fbigp,
            tc.tile_pool(name="work", bufs=4) as workp,
            tc.tile_pool(name="ps_big", bufs=int(os.environ.get("K_PSBIG","5")), space="PSUM") as psbig,
            tc.tile_pool(name="ps_small", bufs=int(os.environ.get("K_PSSMALL","3")), space="PSUM") as pssmall,
            tc.tile_pool(name="dram", bufs=1, space="DRAM") as dramp,
        ):
            # ---- constants into SBUF (two packed DMAs) ----
            CW = cons["cpack"].shape[1]
            cpk = constp.tile([128, CW], F32R, name="cpk")
            nc.sync.dma_start(out=cpk[:, :], in_=cts["cpack"][:, :])
            cpk65 = constp.tile([S, 256], F32, name="cpk65")
            nc.sync.dma_start(out=cpk65[:, :], in_=cts["cpack65"][:, :])

            def cseg(nm, w, f32r=False):
                ap = cpk[:, coff[nm]:coff[nm] + w]
                return ap if f32r else ap.bitcast(F32)

            xcs0 = cseg("xcs0", 256, True)
            xcs1 = cseg("xcs1", 256, True)
            r1_0 = cseg("r10", FW)
            r1_1 = cseg("r11", FW)
            r2_0 = cseg("r20", FW)
            r2_1 = cseg("r21", FW)
            e_t = cseg("e", 128)
            c65t_t = cseg("c65t", S)
            vcsa = cseg("vcsa", 256, True)
            vcsb = cseg("vcsb", 256, True)
            aba = cpk65[:, 0:128]
            abb = cpk65[:, 128:256]

            # ---- big persistent SBUF tensors ----
            EDT = BF16 if USE_BF16 else F32R
            fown = fbigp.tile([N2, C1 * FW], F32R, name="fown")
            fown_e = (fbigp.tile([N2, C1 * FW], BF16, name="fown_e")
                      if USE_BF16 else fown)
            k2 = fbigp.tile([N2, C1 * DL * S], EDT, name="k2")
            fbig = fbigp.tile([N2, B * C1 * FW], EDT, name="fbig")

            # ---- DRAM scratch for the AllGather (c-chunked) ----
            NCH = int(os.environ.get("K_NCHUNKS", "8"))
            CH = C1 // NCH
            f_slice = dramp.tile([C1 * N2, FW], EDT, name="f_slice")
            f_allX_list = [
                dramp.tile([B * CH * N2, FW], EDT, name=f"f_all{h}",
                           addr_space="Shared" if comm else "Local")
                for h in range(NCH)
            ]

            mm = nc.tensor.matmul
            fbig4 = fbig[:, :].rearrange("n (b c r w) -> n b (c r w)",
                                         b=B, c=C1, r=2)

            def fview(c):  # [128, B, 2, 65] for channel c
                return (fbig4[:, :, c * FW:(c + 1) * FW]
                        .rearrange("n b (r w) -> n b r w", r=2))

            def kview(c, d):  # K[c,d] broadcast to [128, B, 2, 65]
                ksl = k2[:, (c * DL + d) * S:(c * DL + d + 1) * S]
                return ksl.unsqueeze(1).unsqueeze(1).broadcast_to((N2, B, 2, S))

            # ---------------- per-phase emitters ----------------
            def forward(c):
                fxt = workp.tile([128, 2 * N1], F32R, tag="fimg",
                                 name=f"fx_{c}", bufs=6)
                nc.sync.dma_start(
                    out=fxt[:, :].rearrange("x (h y) -> x h y", h=2),
                    in_=f_in[c].rearrange("(h x) y -> x h y", h=2))
                fx0 = fxt[:, 0:N1]
                fx1 = fxt[:, N1:2 * N1]
                hts = []
                for h in range(2):
                    psht = psbig.tile([128, 256], F32, tag="big",
                                      name=f"psht{c}_{h}")
                    mm(psht[:, :], fx0[:, h * 128:(h + 1) * 128], xcs0,
                       start=True, stop=False)
                    mm(psht[:, :], fx1[:, h * 128:(h + 1) * 128], xcs1,
                       start=False, stop=True)
                    ht = workp.tile([128, 256], F32, tag="htsb",
                                    name=f"ht{c}_{h}", bufs=6)
                    nc.scalar.copy(ht[:, :], psht[:, :])
                    hts.append(ht)
                psf = pssmall.tile([128, FW], F32, tag="small", name=f"psf{c}")
                mm(psf[:, :], hts[0][:, 0:128], r1_0, start=True, stop=False)
                mm(psf[:, :], hts[1][:, 0:128], r1_1, start=False, stop=False)
                mm(psf[:, :], hts[0][:, 128:256], r2_0, start=False, stop=False)
                mm(psf[:, :], hts[1][:, 128:256], r2_1, start=False, stop=True)
                nc.scalar.copy(fown[:, c * FW:(c + 1) * FW], psf[:, :])
                if USE_BF16:
                    nc.scalar.copy(fown_e[:, c * FW:(c + 1) * FW], psf[:, :])

            def kpath(c):
                krs_t = workp.tile([N2, DL * N2], F32, tag="krs",
                                   name=f"krs{c}", bufs=3)
                nc.sync.dma_start(
                    out=krs_t[:, :].rearrange("u (d v) -> u d v", d=DL),
                    in_=krs_in[c].rearrange("d u v -> u d v"))
                psm1 = pssmall.tile([N2, DL * S], F32, tag="small",
                                    name=f"psm1_{c}")
                for d in range(DL):
                    mm(psm1[:, d * S:(d + 1) * S],
                       krs_t[:, d * N2:(d + 1) * N2], c65t_t,
                       start=True, stop=True)
                m1 = workp.tile([N2, DL * S], F32, tag="m1", name=f"m1_{c}")
                nc.scalar.copy(m1[:, :], psm1[:, :])
                psk = pssmall.tile([N2, DL * S], F32, tag="small",
                                   name=f"psk{c}")
                for d in range(DL):
                    mm(psk[:, d * S:(d + 1) * S], e_t, m1[:, d * S:(d + 1) * S],
                       start=True, stop=True)
                nc.scalar.copy(k2[:, c * DL * S:(c + 1) * DL * S], psk[:, :])

            def gather(half, f_allX):
                c0 = half * CH
                nc.sync.dma_start(
                    out=f_slice[c0 * N2:(c0 + CH) * N2, :]
                        .rearrange("(g n) w -> n g w", n=N2),
                    in_=fown_e[:, c0 * FW:(c0 + CH) * FW]
                        .rearrange("n (g w) -> n g w", w=FW))
                in_ap = f_slice[c0 * N2:(c0 + CH) * N2, :]
                if comm:
                    nc.gpsimd.collective_compute(
                        "AllGather",
                        mybir.AluOpType.bypass,
                        replica_groups=[list(range(NCORES))],
                        ins=[in_ap],
                        outs=[f_allX.opt()],
                    )
                else:
                    for j in range(NCORES):
                        nc.sync.dma_start(
                            out=f_allX[j * CH * N2:(j + 1) * CH * N2, :],
                            in_=in_ap)
                fav = f_allX[:].rearrange("(b c n) w -> n b c w", b=B, n=N2)
                for cl in range(CH):
                    c = c0 + cl
                    nc.sync.dma_start(
                        out=fbig4[:, :, c * FW:(c + 1) * FW],
                        in_=fav[:, :, cl, :],
                    )

            def inverse(src_ap, out_dram_ap, label):
                pstt = psbig.tile([S, 256], F32, tag="big", name=f"pstt_{label}")
                mm(pstt[:, :], src_ap[:, 0:S], vcsa, start=True, stop=False)
                mm(pstt[:, :], src_ap[:, S:FW], vcsb, start=False, stop=True)
                tt = workp.tile([S, 256], F32, tag="ttsb", name=f"tt_{label}")
                nc.scalar.copy(tt[:, :], pstt[:, :])
                pso = pssmall.tile([N2, N2], F32, tag="small",
                                   name=f"pso_{label}")
                mm(pso[:, :], tt[:, 0:N2], aba, start=True, stop=False)
                mm(pso[:, :], tt[:, N2:256], abb, start=False, stop=True)
                osb = workp.tile([N2, N2], F32, tag="osb", name=f"osb_{label}")
                nc.scalar.copy(osb[:, :], pso[:, :])
                nc.sync.dma_start(out=out_dram_ap, in_=osb[:, :])

            # ---------------- einsum chain bookkeeping ----------------
            D2_SPLIT = int(os.environ.get("K_D2SPLIT", "10"))
            accs = {}
            acc2b = None
            started = set()
            if "einsum" not in skip:
                for d in range(DL):
                    accs[d] = workp.tile([N2, B * FW], F32R, tag=f"acc{d}",
                                         name=f"acc{d}", bufs=1)
                acc2b = workp.tile([N2, B * FW], F32R, tag="acc2b",
                                   name="acc2b", bufs=1)
                acc3b = workp.tile([N2, B * FW], F32R, tag="acc3b",
                                   name="acc3b", bufs=1)

            def chain_op(eng, acc, key, d, c):
                ptag = "proda" if eng is nc.vector else "prodb"
                a4 = acc[:, :].rearrange("n (b r w) -> n b r w", b=B, r=2)
                if key not in started:
                    started.add(key)
                    eng.tensor_mul(a4, fview(c), kview(c, d))
                else:
                    prod = workp.tile([N2, B * FW], EDT, tag=ptag,
                                      name=f"prod{d}_{c}", bufs=4)
                    p4 = prod[:, :].rearrange("n (b r w) -> n b r w", b=B, r=2)
                    eng.tensor_mul(p4, fview(c), kview(c, d))
                    eng.tensor_add(acc[:, :], acc[:, :], prod[:, :])

            D3_SPLIT = int(os.environ.get("K_D3SPLIT", "14"))  # c< : GP, c>= : DVE

            def einsum_chunk(cs):
                # GpSimd (slower) gets the early-arriving head ranges; DVE
                # finishes the late tails so the final ops land on the fast
                # engine.
                for c in cs:
                    if c < D3_SPLIT:
                        chain_op(nc.gpsimd, accs[3], "d3", 3, c)
                    if c < D2_SPLIT:
                        chain_op(nc.gpsimd, acc2b, "d2b", 2, c)
                for c in cs:
                    chain_op(nc.vector, accs[0], "d0", 0, c)
                    chain_op(nc.vector, accs[1], "d1", 1, c)
                    if c >= D2_SPLIT:
                        chain_op(nc.vector, accs[2], "d2a", 2, c)
                    if c >= D3_SPLIT:
                        chain_op(nc.vector, acc3b, "d3b", 3, c)

            # ---------------- main flow, chunk-interleaved -------------
            for half, f_allX in enumerate(f_allX_list):
                for cl in range(CH):
                    c = half * CH + cl
                    forward(c)
                    if "kpath" not in skip:
                        kpath(c)
                gather(half, f_allX)
                if "einsum" not in skip:
                    einsum_chunk(range(half * CH, (half + 1) * CH))
                for cl in range(CH if "out1" not in skip else 0):
                    c = half * CH + cl
                    inverse(fown[:, c * FW:(c + 1) * FW], out1_t[c, :, :],
                            f"o1_{c}")

            if "einsum" not in skip and "inv2" not in skip:
                for b in range(B):
                    inverse(accs[0][:, b * FW:(b + 1) * FW],
                            out2_t[b, 0, :, :], f"o2_0_{b}")
                for b in range(B):
                    inverse(accs[1][:, b * FW:(b + 1) * FW],
                            out2_t[b, 1, :, :], f"o2_1_{b}")
                if D3_SPLIT < C1:
                    nc.vector.tensor_add(accs[3][:, :], accs[3][:, :],
                                         acc3b[:, :])
                for b in range(B):
                    inverse(accs[3][:, b * FW:(b + 1) * FW],
                            out2_t[b, 3, :, :], f"o2_3_{b}")
                if D2_SPLIT < C1:
                    nc.vector.tensor_add(accs[2][:, :], accs[2][:, :],
                                         acc2b[:, :])
                    inv2_src = accs[2]
                else:
                    inv2_src = acc2b
                for b in range(B):
                    inverse(inv2_src[:, b * FW:(b + 1) * FW],
                            out2_t[b, 2, :, :], f"o2_2_{b}")

    nc.compile()
    return nc, cons


def _get_program():
    global _PROGRAM
    if _PROGRAM is None:
        _PROGRAM = _build_program()
    return _PROGRAM


def estimate_time(trace_path=None, skip=()):
    """Cost-model makespan of the per-core program (collective as local DMA)."""
    from concourse.timeline_sim import TimelineSim

    nc, _ = _build_program(comm=False, skip=skip)
    ts = TimelineSim(nc, trace=trace_path is not None)
    makespan = ts.simulate()
    if trace_path is not None and ts.perfetto is not None:
        ts.perfetto.save(trace_path)
    return makespan


def kernel(f, kernel):
    """Full inputs in, full outputs out.  f: [8,16,256,256], kernel: [1,16,32,128,128]."""
    global LAST_RESULT
    nc, cons = _get_program()

    f = np.ascontiguousarray(np.asarray(f, dtype=np.float32))
    kr = np.asarray(kernel, dtype=np.float32)[0]                  # [16,32,128,128]
    krs = 0.5 * (kr + np.swapaxes(kr, -1, -2))                    # host symmetrize

    in_maps = []
    for i in range(NCORES):
        m = {"f_in": f[i], "krs_in": np.ascontiguousarray(krs[:, i * DL:(i + 1) * DL])}
        for k, v in cons.items():
            m["c_" + k] = v
        in_maps.append(m)

    res = run_bass_kernel_spmd(nc, in_maps, list(range(NCORES)))
    LAST_RESULT = res

    out1 = np.empty((B, C1, N2, N2), dtype=np.float32)
    out2 = np.empty((B, C2, N2, N2), dtype=np.float32)
    for i in range(NCORES):
        out1[i] = res.results[i]["out1"]
        out2[:, i * DL:(i + 1) * DL] = res.results[i]["out2"]
    return out1, out2

